# revision 9
# baseline (speedup 1.0000x reference)
"""Trainium2 Bass kernel for nn_CustomLSTM_8461085573201.

The reference collapses to out = tanh(inputs[:, -1, :] @ kernel + bias):
only the last timestep of the [B=256, T=512, F=256] input is read, so the
device work is a [256,256] @ [256,256] matmul + bias + tanh.

Sharding over 8 cores: 2D (batch x 4, units x 2). Each core receives one
packed [128, 385] f32 tensor holding, along the free dim:
  [0:64]    x0 = xT[0:128, bs]     (last-timestep slice, transposed)
  [64:128]  x1 = xT[128:256, bs]
  [128:256] w0 = W[0:128, us]
  [256:384] w1 = W[128:256, us]
  [384]     bias[us]
Packing everything into ONE DMA keeps every consumer instruction at a
single DMA-semaphore wait (the Matmult wait-slot budget is tiny) and one
large transfer is also the fastest way to move the bytes (a single
InstDMACopy already fans out across all 16 SDMA engines).

Each core computes outT[u, b] = tanh(sum_f W[f,u] x[b,f] + bias[u]) as two
K=128 PSUM-accumulated matmuls + one ScalarE Tanh (bias is the per-partition
activation bias). Host transposes each core's [128, 64] result and stitches
the full [256, 256] output.
"""

import sys

sys.path.insert(0, "/opt/trn_rl_repo")

import numpy as np

import concourse.bass as bass
import concourse.tile as tile
from concourse import mybir
from concourse.bass_utils import run_bass_kernel_spmd
from concourse.vector_clock import ScopedClock, VectorClock

# The walrus backend in this container allows at most ONE embedded sync wait
# per instruction. Tile's stage-3 kernel-tail emits a single Drain waiting on
# every used proc lane, which walrus rejects. Replace it with a chain of
# single-wait NoOps (one per proc) followed by a waitless drain + the normal
# barriers. The kernel body itself is structured so every instruction has at
# most one cross-engine dependency.


def _split_wait_drain_and_barrier(self, tick_clock, wait_clock):
    nc = self.nc
    gc = tick_clock.global_clock
    for proc in range(len(gc)):
        t = gc[proc]
        if t > 0:
            single = VectorClock()
            single.require_at_least(proc, t)
            w = nc.sync.nop(nofuse=True, hint=f"split_drain_wait_{proc}")
            wait_clock.add_sem_waits(w.ins, ScopedClock({None: single}))
    nc.sync.drain()
    nc.all_engine_barrier()
    assert self.sems is not None
    popped = nc._tile_sem_poison_stack.pop()
    assert popped is self._sem_poison
    nc.clear_and_free_semaphores(list(self.sems.allocated().values()))
    nc.all_engine_barrier()


tile.TileContext._drain_and_barrier = _split_wait_drain_and_barrier

B, T, F, U = 256, 512, 256, 256
N_CORES = 8
RB, CU = 4, 2              # batch split x unit split
BS, US = B // RB, U // CU  # 64, 128
PACK = 2 * BS + 2 * US     # 384
FP32 = mybir.dt.float32

_cached_nc = None


def _build_nc() -> bass.Bass:
    # The walrus backend here allows only ONE embedded sync-wait per compute
    # instruction, so the dataflow is shaped so every instruction depends on
    # at most one cross-engine producer:
    #   dma1 (x+w pack) -> mm1 waits dma1-sem
    #   dma2 (bias|ones row) -> mm_bias waits dma2-sem
    #   mm chain: PE program order
    #   tanh waits PE only (bias added in PSUM via rank-1 matmul bias x ones)
    #   out-dma waits ACT
    nc = bass.Bass()
    data = nc.declare_dram_parameter("data", [128, PACK], FP32, isOutput=False)
    brow = nc.declare_dram_parameter("brow", [1, US + BS], FP32, isOutput=False)
    outT = nc.declare_dram_parameter("outT", [US, BS], FP32, isOutput=True)

    with tile.TileContext(nc) as tc:
        with (
            tc.tile_pool(name="sbuf", bufs=1) as sbuf,
            tc.tile_pool(name="psum", bufs=1, space="PSUM") as psum,
        ):
            d = sbuf.tile([128, PACK], FP32, tag="d")
            d2 = sbuf.tile([1, US + BS], FP32, tag="d2")
            ot = sbuf.tile([US, BS], FP32, tag="ot")
            p = psum.tile([US, BS], FP32)

            nc.sync.dma_start(out=d[:], in_=data[:])
            nc.sync.dma_start(out=d2[:], in_=brow[:])

            x0 = d[:, 0:BS]
            x1 = d[:, BS : 2 * BS]
            w0 = d[:, 2 * BS : 2 * BS + US]
            w1 = d[:, 2 * BS + US : 2 * BS + 2 * US]

            # p[u, b] = bias[u] * 1 + sum_f w[f, u] x[b, f]
            nc.tensor.matmul(
                p[:], d2[0:1, 0:US], d2[0:1, US : US + BS], start=True, stop=False
            )
            nc.tensor.matmul(p[:], w0, x0, start=False, stop=False)
            nc.tensor.matmul(p[:], w1, x1, start=False, stop=True)

            nc.scalar.activation(ot[:], p[:], mybir.ActivationFunctionType.Tanh)
            nc.sync.dma_start(out=outT[:], in_=ot[:])
    return nc


def _get_nc() -> bass.Bass:
    global _cached_nc
    if _cached_nc is None:
        _cached_nc = _build_nc()
    return _cached_nc


def _pack_inputs(inputs, kernel, bias):
    x_last = np.ascontiguousarray(inputs[:, -1, :], dtype=np.float32)  # [B, F]
    xT = np.ascontiguousarray(x_last.T)                                # [F, B]
    w = np.asarray(kernel, dtype=np.float32)
    b = np.asarray(bias, dtype=np.float32)

    in_maps = []
    for core in range(N_CORES):
        bi, ui = divmod(core, CU)
        data = np.empty((128, PACK), dtype=np.float32)
        bs = slice(bi * BS, (bi + 1) * BS)
        us = slice(ui * US, (ui + 1) * US)
        data[:, 0:BS] = xT[0:128, bs]
        data[:, BS : 2 * BS] = xT[128:256, bs]
        data[:, 2 * BS : 2 * BS + US] = w[0:128, us]
        data[:, 2 * BS + US : 2 * BS + 2 * US] = w[128:256, us]
        brow = np.empty((1, US + BS), dtype=np.float32)
        brow[0, 0:US] = b[us]
        brow[0, US:] = 1.0
        in_maps.append({"data": data, "brow": brow})
    return in_maps


def kernel(inputs: np.ndarray, kernel: np.ndarray, bias: np.ndarray) -> np.ndarray:
    in_maps = _pack_inputs(inputs, kernel, bias)
    res = run_bass_kernel_spmd(_get_nc(), in_maps, list(range(N_CORES)))

    out = np.empty((B, U), dtype=np.float32)
    for core in range(N_CORES):
        bi, ui = divmod(core, CU)
        out[bi * BS : (bi + 1) * BS, ui * US : (ui + 1) * US] = res.results[core][
            "outT"
        ].T
    return out
